# revision 4
# baseline (speedup 1.0000x reference)
"""Trainium2 Bass kernel for nn_CrossAttn_65214783422649.

Key algebraic reduction: softmax over R followed by mean over R is identically
1/R, so the whole attention branch (Wq, Wk, energy, softmax) cancels out of
the output:

    sims[i, c] = (a_c + b_i) . cap_vec_c / ||a_c + b_i||
      a_c      = (gamma/R) * sum_t mask * leaky(cap_c @ Wvt.T + bvt)
      b_i      = mean_r leaky(img_i @ Wvi.T + bvi)
      cap_vec  = l2norm(masked-mean_t cap_c)

Sharding: captions (B_c) 8-way; images also 8-way for the b_i computation,
with an on-chip AllGather of the [16, 1024] b shards.  Big matmuls run in
fp32r (full PE rate); the tiny similarity-assembly matmuls run in fp32.
"""

import numpy as np

import concourse.bass as bass
import concourse.mybir as mybir
import concourse.tile as tile
from concourse import bacc
from concourse.bass import ds, ts
from concourse.bass_utils import run_bass_kernel_spmd
from concourse.masks import make_identity

F32 = mybir.dt.float32
F32R = mybir.dt.float32r
AF = mybir.ActivationFunctionType

N_CORES = 8
B_I, B_C, R, T, D = 128, 128, 36, 64, 1024
C_SH = B_C // N_CORES          # 16 captions per core
I_SH = B_I // N_CORES          # 16 images per core
CAP_TOK = C_SH * T             # 1024 caption tokens per core
IMG_TOK = I_SH * R             # 576 image tokens per core
IMG_PAD = 640                  # padded to 5 * 128
KT = D // 128                  # 8 contraction tiles
CT = CAP_TOK // 128            # 8 caption token tiles
IT = IMG_PAD // 128            # 5 image token tiles
NEG_SLOPE = 0.1

_CACHE: dict = {}


def _build():
    nc = bacc.Bacc("TRN2", target_bir_lowering=False, debug=False,
                   num_devices=N_CORES)

    cap_d = nc.dram_tensor("cap", [CAP_TOK, D], F32, kind="ExternalInput")
    img_d = nc.dram_tensor("img", [IMG_PAD, D], F32, kind="ExternalInput")
    wvt_d = nc.dram_tensor("wvt", [D, D], F32, kind="ExternalInput")
    wvi_d = nc.dram_tensor("wvi", [D, D], F32, kind="ExternalInput")
    bvt_d = nc.dram_tensor("bvt_row", [1, D], F32, kind="ExternalInput")
    bvi_d = nc.dram_tensor("bvi_row", [1, D], F32, kind="ExternalInput")
    maskv_d = nc.dram_tensor("maskv", [CAP_TOK, 1], F32, kind="ExternalInput")
    om_a_d = nc.dram_tensor("om_a", [CAP_TOK, C_SH], F32, kind="ExternalInput")
    om_cm_d = nc.dram_tensor("om_cm", [CAP_TOK, C_SH], F32, kind="ExternalInput")
    om_b_d = nc.dram_tensor("om_b", [IMG_PAD, I_SH], F32, kind="ExternalInput")
    sims_d = nc.dram_tensor("sims", [C_SH, B_I], F32, kind="ExternalOutput")

    with tile.TileContext(nc) as tc:
        with (
            tc.tile_pool(name="const", bufs=1) as const,
            tc.tile_pool(name="wt", bufs=1) as wtp,
            tc.tile_pool(name="nat", bufs=3) as natp,
            tc.tile_pool(name="xt", bufs=2) as xtp,
            tc.tile_pool(name="vtx", bufs=2) as vtxp,
            tc.tile_pool(name="gpool", bufs=1) as gp,
            tc.tile_pool(name="small", bufs=1) as sp,
            tc.tile_pool(name="ps_tr", bufs=2, space="PSUM") as ps_tr,
            tc.tile_pool(name="ps_mm", bufs=2, space="PSUM") as ps_mm,
            tc.tile_pool(name="ps_acc", bufs=1, space="PSUM") as ps_acc,
            tc.tile_pool(name="dram", bufs=1, space="DRAM") as dram,
        ):
            # ---- constants
            ident32 = const.tile([128, 128], F32, tag="id32")
            make_identity(nc, ident32[:])
            identr = const.tile([128, 128], F32R, tag="idr")
            nc.vector.tensor_copy(identr[:], ident32[:])

            ones_row32 = const.tile([1, 128], F32, tag="ones_row32")
            nc.vector.memset(ones_row32[:], 1.0)
            ones_row_r = const.tile([1, 128], F32R, tag="ones_rowr")
            nc.vector.tensor_copy(ones_row_r[:], ones_row32[:])
            ones_col32 = const.tile([128, 1], F32, tag="ones_col32")
            nc.vector.memset(ones_col32[:], 1.0)

            bvt_r = const.tile([1, D], F32R, tag="bvt")
            nc.gpsimd.dma_start(out=bvt_r[:], in_=bvt_d[:, :])
            bvi_r = const.tile([1, D], F32R, tag="bvi")
            nc.gpsimd.dma_start(out=bvi_r[:], in_=bvi_d[:, :])

            om_a_s = const.tile([128, CT, C_SH], F32R, tag="om_a")
            nc.gpsimd.dma_start(
                out=om_a_s[:], in_=om_a_d.rearrange("(a p) c -> p a c", p=128))
            om_cm_s = const.tile([128, CT, C_SH], F32R, tag="om_cm")
            nc.gpsimd.dma_start(
                out=om_cm_s[:], in_=om_cm_d.rearrange("(a p) c -> p a c", p=128))
            om_b_s = const.tile([128, IT, I_SH], F32R, tag="om_b")
            nc.gpsimd.dma_start(
                out=om_b_s[:], in_=om_b_d.rearrange("(a p) c -> p a c", p=128))
            maskv_s = const.tile([128, CT], F32, tag="maskv")
            nc.sync.dma_start(
                out=maskv_s[:], in_=maskv_d.rearrange("(a p) c -> p (a c)", p=128))

            # ---- weight transposes: W [D, D] -> WT [k_p, d_f] (fp32r)
            wt_vi = wtp.tile([128, KT, D], F32R, tag="wt_vi")
            wt_vt = wtp.tile([128, KT, D], F32R, tag="wt_vt")
            for w_d, wt_s in ((wvi_d, wt_vi), (wvt_d, wt_vt)):
                for k in range(KT):
                    wnat = natp.tile([128, KT, 128], F32R, tag="nat")
                    nc.gpsimd.dma_start(
                        out=wnat[:],
                        in_=w_d[:, ts(k, 128)].rearrange("(a p) c -> p a c", p=128))
                    for db in range(KT):
                        pst = ps_tr.tile([128, 128], F32R, tag="tr")
                        nc.tensor.transpose(pst[:], wnat[:, db, :], identr[:])
                        if db % 2 == 0:
                            nc.scalar.activation(wt_s[:, k, ts(db, 128)], pst[:], AF.Copy)
                        else:
                            nc.vector.tensor_copy(wt_s[:, k, ts(db, 128)], pst[:])

            # ---- image phase: b_i shard, then AllGather
            ps_b = [ps_acc.tile([I_SH, 512], F32, tag=f"acc{dh}", name=f"ps_b{dh}") for dh in range(2)]
            for it in range(IT):
                inat = natp.tile([128, D], F32R, tag="nat")
                nc.gpsimd.dma_start(out=inat[:], in_=img_d[ts(it, 128), :])
                xT = xtp.tile([128, KT, 128], F32R, tag="xt")
                for k in range(KT):
                    pst = ps_tr.tile([128, 128], F32R, tag="tr")
                    nc.tensor.transpose(pst[:], inat[:, ts(k, 128)], identr[:])
                    if k % 2 == 0:
                        nc.scalar.activation(xT[:, k, :], pst[:], AF.Copy)
                    else:
                        nc.vector.tensor_copy(xT[:, k, :], pst[:])
                vimg = vtxp.tile([128, D], F32R, tag="vtx")
                for dh in range(2):
                    pm = ps_mm.tile([128, 512], F32, tag="mm")
                    for k in range(KT):
                        nc.tensor.matmul(pm[:], xT[:, k, :],
                                         wt_vi[:, k, ds(dh * 512, 512)],
                                         start=(k == 0), stop=False)
                    nc.tensor.matmul(pm[:], ones_row_r[:, 0:128],
                                     bvi_r[:, ds(dh * 512, 512)],
                                     start=False, stop=True)
                    nc.scalar.activation(vimg[:, ds(dh * 512, 512)], pm[:],
                                         AF.Prelu, alpha=NEG_SLOPE)
                    nc.tensor.matmul(ps_b[dh][:], om_b_s[:, it, :],
                                     vimg[:, ds(dh * 512, 512)],
                                     start=(it == 0), stop=(it == IT - 1))
            bnat = sp.tile([I_SH, D], F32, tag="bnat")
            for dh in range(2):
                nc.scalar.activation(bnat[:, ds(dh * 512, 512)], ps_b[dh][:], AF.Copy)
            ag_in = dram.tile([I_SH, D], F32, tag="ag_in")
            ag_out = dram.tile([B_I, D], F32, addr_space="Shared", tag="ag_out")
            nc.sync.dma_start(out=ag_in[:], in_=bnat[:])
            nc.gpsimd.collective_compute(
                "AllGather",
                mybir.AluOpType.bypass,
                replica_groups=[list(range(N_CORES))],
                ins=[ag_in[:].opt()],
                outs=[ag_out[:].opt()],
            )

            # ---- caption phase: a_c and capsum_c
            ps_a = [ps_acc.tile([C_SH, 512], F32, tag=f"acc{dh}", name=f"ps_a{dh}") for dh in range(2)]
            ps_cm = [ps_acc.tile([C_SH, 512], F32, tag=f"acc{dh+2}", name=f"ps_cm{dh}") for dh in range(2)]
            for ct in range(CT):
                cnat = natp.tile([128, D], F32R, tag="nat")
                nc.gpsimd.dma_start(out=cnat[:], in_=cap_d[ts(ct, 128), :])
                xT = xtp.tile([128, KT, 128], F32R, tag="xt")
                for k in range(KT):
                    pst = ps_tr.tile([128, 128], F32R, tag="tr")
                    nc.tensor.transpose(pst[:], cnat[:, ts(k, 128)], identr[:])
                    if k % 2 == 0:
                        nc.scalar.activation(xT[:, k, :], pst[:], AF.Copy)
                    else:
                        nc.vector.tensor_copy(xT[:, k, :], pst[:])
                vtxt = vtxp.tile([128, D], F32R, tag="vtx")
                for dh in range(2):
                    pm = ps_mm.tile([128, 512], F32, tag="mm")
                    for k in range(KT):
                        nc.tensor.matmul(pm[:], xT[:, k, :],
                                         wt_vt[:, k, ds(dh * 512, 512)],
                                         start=(k == 0), stop=False)
                    nc.tensor.matmul(pm[:], ones_row_r[:, 0:128],
                                     bvt_r[:, ds(dh * 512, 512)],
                                     start=False, stop=True)
                    nc.scalar.activation(vtxt[:, ds(dh * 512, 512)], pm[:],
                                         AF.Prelu, scale=maskv_s[:, ct:ct + 1],
                                         alpha=NEG_SLOPE)
                    nc.tensor.matmul(ps_a[dh][:], om_a_s[:, ct, :],
                                     vtxt[:, ds(dh * 512, 512)],
                                     start=(ct == 0), stop=(ct == CT - 1))
                    nc.tensor.matmul(ps_cm[dh][:], om_cm_s[:, ct, :],
                                     cnat[:, ds(dh * 512, 512)],
                                     start=(ct == 0), stop=(ct == CT - 1))
            a_s = sp.tile([C_SH, D], F32, tag="a_s")
            cs_s = sp.tile([C_SH, D], F32, tag="cs_s")
            for dh in range(2):
                nc.scalar.activation(a_s[:, ds(dh * 512, 512)], ps_a[dh][:], AF.Copy)
                nc.scalar.activation(cs_s[:, ds(dh * 512, 512)], ps_cm[dh][:], AF.Copy)

            # ---- similarity assembly (all fp32)
            bfull = gp.tile([B_I, D], F32, tag="bfull")
            nc.sync.dma_start(out=bfull[:], in_=ag_out[:])

            bT = gp.tile([128, KT, B_I], F32, tag="bT")
            for k in range(KT):
                pst = ps_tr.tile([128, 128], F32, tag="tr")
                nc.tensor.transpose(pst[:], bfull[:, ts(k, 128)], ident32[:])
                nc.vector.tensor_copy(bT[:, k, :], pst[:])
            bT2 = gp.tile([128, KT, B_I], F32, tag="bT2")
            nc.vector.tensor_scalar_mul(bT2[:], bT[:], 2.0)

            bsq = gp.tile([B_I, D], F32, tag="bsq")
            nc.vector.tensor_mul(bsq[:], bfull[:], bfull[:])
            nb_col = sp.tile([B_I, 1], F32, tag="nb_col")
            nc.vector.reduce_sum(nb_col[:], bsq[:], axis=mybir.AxisListType.X)
            ps_nbt = ps_tr.tile([1, 128], F32, tag="tr")
            nc.tensor.transpose(ps_nbt[:], nb_col[:], ident32[:])
            nb_row = sp.tile([1, B_I], F32, tag="nb_row")
            nc.vector.tensor_copy(nb_row[:], ps_nbt[:])

            aT = gp.tile([128, KT, C_SH], F32, tag="aT")
            csT = gp.tile([128, KT, C_SH], F32, tag="csT")
            for src, dst in ((a_s, aT), (cs_s, csT)):
                for k in range(KT):
                    pst = ps_tr.tile([128, C_SH], F32, tag="tr")
                    nc.tensor.transpose(pst[:], src[:, ts(k, 128)],
                                        ident32[0:C_SH, 0:C_SH])
                    nc.vector.tensor_copy(dst[:, k, :], pst[:])

            # pack columns at 0/32/64 so output rows land on legal base partitions
            pack = gp.tile([128, 80], F32, tag="pack")
            ps_sc = ps_acc.tile([80, 1], F32, tag="acc0")
            for k in range(KT):
                nc.vector.tensor_mul(pack[:, 0:C_SH], aT[:, k, :], csT[:, k, :])
                nc.vector.tensor_mul(pack[:, 32:32 + C_SH], aT[:, k, :], aT[:, k, :])
                nc.vector.tensor_mul(pack[:, 64:64 + C_SH], csT[:, k, :], csT[:, k, :])
                nc.tensor.matmul(ps_sc[:], pack[:], ones_col32[:],
                                 start=(k == 0), stop=(k == KT - 1))
            sc_s = sp.tile([80, 1], F32, tag="sc_s")
            nc.vector.tensor_copy(sc_s[:], ps_sc[:])

            sqq = sp.tile([C_SH, 1], F32, tag="sqq")
            nc.scalar.activation(sqq[:], sc_s[64:64 + C_SH, :], AF.Sqrt)
            shat = sp.tile([C_SH, 1], F32, tag="shat")
            nc.vector.reciprocal(shat[:], sqq[:])

            ps_g1 = ps_acc.tile([C_SH, B_I], F32, tag="acc1")
            for k in range(KT):
                nc.tensor.matmul(ps_g1[:], aT[:, k, :], bT2[:, k, :],
                                 start=(k == 0), stop=False)
            nc.tensor.matmul(ps_g1[:], ones_row32[:, 0:C_SH], nb_row[:, :],
                             start=False, stop=True)
            den = sp.tile([C_SH, B_I], F32, tag="den")
            nc.scalar.activation(den[:], ps_g1[:], AF.Sqrt,
                                 bias=sc_s[32:32 + C_SH, :])
            rden = sp.tile([C_SH, B_I], F32, tag="rden")
            nc.vector.reciprocal(rden[:], den[:])

            ps_g2 = ps_acc.tile([C_SH, B_I], F32, tag="acc2")
            for k in range(KT):
                nc.tensor.matmul(ps_g2[:], csT[:, k, :], bT[:, k, :],
                                 start=(k == 0), stop=(k == KT - 1))
            num = sp.tile([C_SH, B_I], F32, tag="num")
            nc.vector.tensor_scalar(
                out=num[:], in0=ps_g2[:], scalar1=sc_s[0:C_SH, :],
                scalar2=shat[:], op0=mybir.AluOpType.add,
                op1=mybir.AluOpType.mult)
            sims_s = sp.tile([C_SH, B_I], F32, tag="sims_s")
            nc.vector.tensor_mul(sims_s[:], num[:], rden[:])
            nc.sync.dma_start(out=sims_d[:, :], in_=sims_s[:])

    nc.compile()
    return nc


def _get_nc():
    if "nc" not in _CACHE:
        _CACHE["nc"] = _build()
    return _CACHE["nc"]


def _host_prep(inputs):
    cap_embed = np.asarray(inputs["cap_embed"], dtype=np.float32)
    img_embed = np.asarray(inputs["img_embed"], dtype=np.float32)
    lens = np.asarray(inputs["lens"]).astype(np.int32)
    wvt = np.ascontiguousarray(np.asarray(inputs["Wvt"], dtype=np.float32))
    wvi = np.ascontiguousarray(np.asarray(inputs["Wvi"], dtype=np.float32))
    bvt = np.asarray(inputs["bvt"], dtype=np.float32).reshape(1, D)
    bvi = np.asarray(inputs["bvi"], dtype=np.float32).reshape(1, D)
    gamma = float(np.asarray(inputs["gamma_img"]).reshape(-1)[0])

    mask_full = (np.arange(T)[None, :] < lens[:, None]).astype(np.float32)  # [B_C, T]

    blk_cap = np.repeat(np.eye(C_SH, dtype=np.float32), T, axis=0)  # [1024, 16]
    om_b = np.zeros((IMG_PAD, I_SH), np.float32)
    om_b[:IMG_TOK] = np.repeat(np.eye(I_SH, dtype=np.float32), R, axis=0) / R

    in_maps = []
    for m in range(N_CORES):
        cs = slice(m * C_SH, (m + 1) * C_SH)
        im = slice(m * I_SH, (m + 1) * I_SH)
        msk = mask_full[cs]                                      # [16, T]
        cap = (cap_embed[cs] * msk[:, :, None]).reshape(CAP_TOK, D)
        img = np.zeros((IMG_PAD, D), np.float32)
        img[:IMG_TOK] = img_embed[im].reshape(IMG_TOK, D)
        in_maps.append({
            "cap": np.ascontiguousarray(cap),
            "img": img,
            "wvt": wvt,
            "wvi": wvi,
            "bvt_row": bvt,
            "bvi_row": bvi,
            "maskv": np.ascontiguousarray(msk.reshape(CAP_TOK, 1)),
            "om_a": np.ascontiguousarray(blk_cap * (gamma / R)),
            "om_cm": blk_cap,
            "om_b": om_b,
        })
    return in_maps


def kernel(**inputs) -> np.ndarray:
    nc = _get_nc()
    in_maps = _host_prep(inputs)
    res = run_bass_kernel_spmd(nc, in_maps, core_ids=list(range(N_CORES)))
    sims = np.empty((B_I, B_C), np.float32)
    for m in range(N_CORES):
        sims[:, m * C_SH:(m + 1) * C_SH] = res.results[m]["sims"].T
    return sims


def run_traced(**inputs):
    """For test.py: same as kernel() but with NTFF tracing enabled."""
    nc = _get_nc()
    in_maps = _host_prep(inputs)
    res = run_bass_kernel_spmd(nc, in_maps, core_ids=list(range(N_CORES)),
                               trace=True)
    sims = np.empty((B_I, B_C), np.float32)
    for m in range(N_CORES):
        sims[:, m * C_SH:(m + 1) * C_SH] = res.results[m]["sims"].T
    return sims, res


# revision 7
# speedup vs baseline: 1.0459x; 1.0459x over previous
"""Trainium2 Bass kernel for nn_CrossAttn_65214783422649.

Key algebraic reduction: softmax over R followed by mean over R is identically
1/R, so the whole attention branch (Wq, Wk, energy, softmax) cancels out of
the output:

    sims[i, c] = (a_c + b_i) . cap_vec_c / ||a_c + b_i||
      a_c      = (gamma/R) * sum_t mask * leaky(cap_c @ Wvt.T + bvt)
      b_i      = mean_r leaky(img_i @ Wvi.T + bvi)
      cap_vec  = l2norm(masked-mean_t cap_c)

Sharding: captions (B_c) 8-way; images also 8-way for the b_i computation,
with an on-chip AllGather of the b shards (+ their squared norms).
Valid caption tokens are host-packed (ragged lens), shrinking the caption
side by ~40%.  Big matmuls run in fp32r (full PE rate); the tiny
similarity-assembly matmuls run in fp32.
"""

import numpy as np

import concourse.bass as bass
import concourse.mybir as mybir
import concourse.tile as tile
from concourse import bacc
from concourse.bass import ds, ts
from concourse.bass_utils import run_bass_kernel_spmd
from concourse.masks import make_identity

F32 = mybir.dt.float32
F32R = mybir.dt.float32r
AF = mybir.ActivationFunctionType

N_CORES = 8
B_I, B_C, R, T, D = 128, 128, 36, 64, 1024
C_SH = B_C // N_CORES          # 16 captions per core
I_SH = B_I // N_CORES          # 16 images per core
IMG_TOK = I_SH * R             # 576 image tokens per core
IMG_PAD = 640                  # padded to 5 * 128
KT = D // 128                  # 8 contraction tiles
IT = IMG_PAD // 128            # 5 image token tiles
NEG_SLOPE = 0.1
AGW = D + 1                    # AllGather row width: b row + |b|^2

_CACHE: dict = {}


def _build(CT: int):
    """CT = number of 128-token caption tiles after host packing."""
    CAP_TOK = CT * 128
    nc = bacc.Bacc("TRN2", target_bir_lowering=False, debug=False,
                   num_devices=N_CORES)

    cap_d = nc.dram_tensor("cap", [CAP_TOK, D], F32, kind="ExternalInput")
    img_d = nc.dram_tensor("img", [IMG_PAD, D], F32, kind="ExternalInput")
    wvt_d = nc.dram_tensor("wvt", [D, D], F32, kind="ExternalInput")
    wvi_d = nc.dram_tensor("wvi", [D, D], F32, kind="ExternalInput")
    bvt_d = nc.dram_tensor("bvt_row", [1, D], F32, kind="ExternalInput")
    bvi_d = nc.dram_tensor("bvi_row", [1, D], F32, kind="ExternalInput")
    om_a_d = nc.dram_tensor("om_a", [CAP_TOK, C_SH], F32, kind="ExternalInput")
    om_cm_d = nc.dram_tensor("om_cm", [CAP_TOK, C_SH], F32, kind="ExternalInput")
    om_b_d = nc.dram_tensor("om_b", [IMG_PAD, I_SH], F32, kind="ExternalInput")
    sims_d = nc.dram_tensor("sims", [C_SH, B_I], F32, kind="ExternalOutput")

    with tile.TileContext(nc) as tc:
        with (
            tc.tile_pool(name="const", bufs=1) as const,
            tc.tile_pool(name="wt", bufs=1) as wtp,
            tc.tile_pool(name="nat", bufs=3) as natp,
            tc.tile_pool(name="capnat", bufs=1) as capnatp,
            tc.tile_pool(name="xt", bufs=1) as xtp,
            tc.tile_pool(name="vtx", bufs=2) as vtxp,
            tc.tile_pool(name="gpool", bufs=1) as gp,
            tc.tile_pool(name="small", bufs=1) as sp,
            tc.tile_pool(name="ps_tr", bufs=2, space="PSUM") as ps_tr,
            tc.tile_pool(name="ps_mm", bufs=2, space="PSUM") as ps_mm,
            tc.tile_pool(name="ps_acc", bufs=1, space="PSUM") as ps_acc,
            tc.tile_pool(name="dram", bufs=1, space="DRAM") as dram,
        ):
            # ---- constants
            ident32 = const.tile([128, 128], F32, tag="id32")
            make_identity(nc, ident32[:])

            ones_row32 = const.tile([1, 128], F32, tag="ones_row32")
            nc.vector.memset(ones_row32[:], 1.0)
            ones_row_r = const.tile([1, 128], F32R, tag="ones_rowr")
            nc.vector.tensor_copy(ones_row_r[:], ones_row32[:])
            ones_col32 = const.tile([128, 1], F32, tag="ones_col32")
            nc.vector.memset(ones_col32[:], 1.0)

            bvt_r = const.tile([1, D], F32R, tag="bvt")
            nc.gpsimd.dma_start(out=bvt_r[:], in_=bvt_d[:, :])
            bvi_r = const.tile([1, D], F32R, tag="bvi")
            nc.gpsimd.dma_start(out=bvi_r[:], in_=bvi_d[:, :])

            om_a_s = const.tile([128, CT, C_SH], F32R, tag="om_a")
            nc.gpsimd.dma_start(
                out=om_a_s[:], in_=om_a_d.rearrange("(a p) c -> p a c", p=128))
            om_cm_s = const.tile([128, CT, C_SH], F32R, tag="om_cm")
            nc.gpsimd.dma_start(
                out=om_cm_s[:], in_=om_cm_d.rearrange("(a p) c -> p a c", p=128))
            om_b_s = const.tile([128, IT, I_SH], F32R, tag="om_b")
            nc.gpsimd.dma_start(
                out=om_b_s[:], in_=om_b_d.rearrange("(a p) c -> p a c", p=128))

            def transpose_in(nat_ap, dst_r, k, nm):
                """PE-transpose fp32 [128,128] slab k of nat_ap into f32r dst."""
                pst = ps_tr.tile([128, 128], F32, tag="tr", name=f"pst{nm}")
                nc.tensor.transpose(pst[:], nat_ap[:, ts(k, 128)], ident32[:])
                if k % 2 == 0:
                    nc.scalar.activation(dst_r[:, k, :], pst[:], AF.Copy)
                else:
                    nc.vector.tensor_copy(dst_r[:, k, :], pst[:])

            # ---- P1: img loads + transposes (imgT resident, f32r)
            imgT = []
            for it in range(IT):
                inat = natp.tile([128, D], F32, tag="nat", name=f"inat{it}")
                nc.sync.dma_start(out=inat[:], in_=img_d[ts(it, 128), :])
                xT = xtp.tile([128, KT, 128], F32R, tag=f"imgT{it}",
                              name=f"imgT{it}")
                for k in range(KT):
                    transpose_in(inat, xT, k, f"i{it}{k}")
                imgT.append(xT)

            # ---- P2: Wvi loads + transposes
            wt_vi = wtp.tile([128, KT, D], F32R, tag="wt_vi")
            for k in range(KT):
                wnat = natp.tile([128, KT, 128], F32, tag="nat", name=f"wvin{k}")
                nc.sync.dma_start(
                    out=wnat[:],
                    in_=wvi_d[:, ts(k, 128)].rearrange("(a p) c -> p a c", p=128))
                for db in range(KT):
                    pst = ps_tr.tile([128, 128], F32, tag="tr", name=f"pw{k}{db}")
                    nc.tensor.transpose(pst[:], wnat[:, db, :], ident32[:])
                    if db % 2 == 0:
                        nc.scalar.activation(wt_vi[:, k, ts(db, 128)], pst[:], AF.Copy)
                    else:
                        nc.vector.tensor_copy(wt_vi[:, k, ts(db, 128)], pst[:])

            # ---- P3: img matmul phase -> b shard + |b|^2, AllGather
            ps_b = [ps_acc.tile([I_SH, 512], F32, tag=f"acc{dh}", name=f"ps_b{dh}")
                    for dh in range(2)]
            for it in range(IT):
                vimg = vtxp.tile([128, D], F32R, tag="vtx", name=f"vimg{it}")
                for dh in range(2):
                    pm = ps_mm.tile([128, 512], F32, tag="mm", name=f"pmi{it}{dh}")
                    for k in range(KT):
                        nc.tensor.matmul(pm[:], imgT[it][:, k, :],
                                         wt_vi[:, k, ds(dh * 512, 512)],
                                         start=(k == 0), stop=False)
                    nc.tensor.matmul(pm[:], ones_row_r[:, 0:128],
                                     bvi_r[:, ds(dh * 512, 512)],
                                     start=False, stop=True)
                    nc.scalar.activation(vimg[:, ds(dh * 512, 512)], pm[:],
                                         AF.Prelu, alpha=NEG_SLOPE)
                    nc.tensor.matmul(ps_b[dh][:], om_b_s[:, it, :],
                                     vimg[:, ds(dh * 512, 512)],
                                     start=(it == 0), stop=(it == IT - 1))
            bnat = sp.tile([I_SH, AGW], F32, tag="bnat")
            for dh in range(2):
                nc.scalar.activation(bnat[:, ds(dh * 512, 512)], ps_b[dh][:], AF.Copy)
            bsq_sh = sp.tile([I_SH, D], F32, tag="bsq_sh")
            nc.vector.tensor_mul(bsq_sh[:], bnat[:, 0:D], bnat[:, 0:D])
            nc.vector.reduce_sum(bnat[:, D:D + 1], bsq_sh[:],
                                 axis=mybir.AxisListType.X)
            ag_in = dram.tile([I_SH, AGW], F32, tag="ag_in")
            ag_out = dram.tile([B_I, AGW], F32, addr_space="Shared", tag="ag_out")
            nc.sync.dma_start(out=ag_in[:], in_=bnat[:])
            nc.gpsimd.collective_compute(
                "AllGather",
                mybir.AluOpType.bypass,
                replica_groups=[list(range(N_CORES))],
                ins=[ag_in[:].opt()],
                outs=[ag_out[:].opt()],
            )

            # ---- P4: cap loads + transposes (capT resident)
            capT = []
            cnats_r = []
            for ct in range(CT):
                cnat = capnatp.tile([128, D], F32, tag="capnat",
                                    name=f"cnat{ct}", bufs=2)
                nc.sync.dma_start(out=cnat[:], in_=cap_d[ts(ct, 128), :])
                xT = xtp.tile([128, KT, 128], F32R, tag=f"capT{ct}",
                              name=f"capT{ct}")
                for k in range(KT):
                    transpose_in(cnat, xT, k, f"c{ct}{k}")
                cnat_r = capnatp.tile([128, D], F32R, tag=f"capnat_r{ct}",
                                      name=f"cnat_r{ct}")
                nc.vector.tensor_copy(cnat_r[:], cnat[:])
                capT.append(xT)
                cnats_r.append(cnat_r)

            # ---- P5: Wvt loads + transposes
            wt_vt = wtp.tile([128, KT, D], F32R, tag="wt_vt")
            for k in range(KT):
                wnat = natp.tile([128, KT, 128], F32, tag="nat", name=f"wvtn{k}")
                nc.sync.dma_start(
                    out=wnat[:],
                    in_=wvt_d[:, ts(k, 128)].rearrange("(a p) c -> p a c", p=128))
                for db in range(KT):
                    pst = ps_tr.tile([128, 128], F32, tag="tr", name=f"pv{k}{db}")
                    nc.tensor.transpose(pst[:], wnat[:, db, :], ident32[:])
                    if db % 2 == 0:
                        nc.scalar.activation(wt_vt[:, k, ts(db, 128)], pst[:], AF.Copy)
                    else:
                        nc.vector.tensor_copy(wt_vt[:, k, ts(db, 128)], pst[:])

            # ---- P6: cap matmul phase -> a, capsum
            ps_a = [ps_acc.tile([C_SH, 512], F32, tag=f"acc{dh}", name=f"ps_a{dh}")
                    for dh in range(2)]
            ps_cm = [ps_acc.tile([C_SH, 512], F32, tag=f"acc{dh+2}", name=f"ps_cm{dh}")
                     for dh in range(2)]
            for ct in range(CT):
                vtxt = vtxp.tile([128, D], F32R, tag="vtx", name=f"vtxt{ct}")
                for dh in range(2):
                    pm = ps_mm.tile([128, 512], F32, tag="mm", name=f"pmc{ct}{dh}")
                    for k in range(KT):
                        nc.tensor.matmul(pm[:], capT[ct][:, k, :],
                                         wt_vt[:, k, ds(dh * 512, 512)],
                                         start=(k == 0), stop=False)
                    nc.tensor.matmul(pm[:], ones_row_r[:, 0:128],
                                     bvt_r[:, ds(dh * 512, 512)],
                                     start=False, stop=True)
                    nc.scalar.activation(vtxt[:, ds(dh * 512, 512)], pm[:],
                                         AF.Prelu, alpha=NEG_SLOPE)
                    nc.tensor.matmul(ps_a[dh][:], om_a_s[:, ct, :],
                                     vtxt[:, ds(dh * 512, 512)],
                                     start=(ct == 0), stop=(ct == CT - 1))
                    nc.tensor.matmul(ps_cm[dh][:], om_cm_s[:, ct, :],
                                     cnats_r[ct][:, ds(dh * 512, 512)],
                                     start=(ct == 0), stop=(ct == CT - 1))
            a_s = sp.tile([C_SH, D], F32, tag="a_s")
            cs_s = sp.tile([C_SH, D], F32, tag="cs_s")
            for dh in range(2):
                nc.scalar.activation(a_s[:, ds(dh * 512, 512)], ps_a[dh][:], AF.Copy)
                nc.scalar.activation(cs_s[:, ds(dh * 512, 512)], ps_cm[dh][:], AF.Copy)

            # ---- P7: similarity assembly (fp32)
            bfull = gp.tile([B_I, D], F32, tag="bfull")
            nc.sync.dma_start(out=bfull[:], in_=ag_out[:, 0:D])
            nb_col = sp.tile([B_I, 1], F32, tag="nb_col")
            nc.sync.dma_start(out=nb_col[:], in_=ag_out[:, D:D + 1])

            bT = gp.tile([128, KT, B_I], F32, tag="bT")
            for k in range(KT):
                pst = ps_tr.tile([128, 128], F32, tag="tr", name=f"pb{k}")
                nc.tensor.transpose(pst[:], bfull[:, ts(k, 128)], ident32[:])
                nc.vector.tensor_copy(bT[:, k, :], pst[:])

            ps_nbt = ps_tr.tile([1, 128], F32, tag="tr", name="ps_nbt")
            nc.tensor.transpose(ps_nbt[:], nb_col[:], ident32[:])
            nb_row = sp.tile([1, B_I], F32, tag="nb_row")
            nc.vector.tensor_copy(nb_row[:], ps_nbt[:])

            aT = gp.tile([128, KT, C_SH], F32, tag="aT")
            csT = gp.tile([128, KT, C_SH], F32, tag="csT")
            for src, dst, nm in ((a_s, aT, "a"), (cs_s, csT, "c")):
                for k in range(KT):
                    pst = ps_tr.tile([128, C_SH], F32, tag="tr", name=f"pq{nm}{k}")
                    nc.tensor.transpose(pst[:], src[:, ts(k, 128)],
                                        ident32[0:C_SH, 0:C_SH])
                    nc.vector.tensor_copy(dst[:, k, :], pst[:])
            aT2 = gp.tile([128, KT, C_SH], F32, tag="aT2")
            nc.vector.tensor_scalar_mul(aT2[:], aT[:], 2.0)

            # pack columns at 0/32/64 so output rows land on legal base partitions
            pack = gp.tile([128, 80], F32, tag="pack")
            ps_sc = ps_acc.tile([80, 1], F32, tag="acc0")
            for k in range(KT):
                nc.vector.tensor_mul(pack[:, 0:C_SH], aT[:, k, :], csT[:, k, :])
                nc.vector.tensor_mul(pack[:, 32:32 + C_SH], aT[:, k, :], aT[:, k, :])
                nc.vector.tensor_mul(pack[:, 64:64 + C_SH], csT[:, k, :], csT[:, k, :])
                nc.tensor.matmul(ps_sc[:], pack[:], ones_col32[:],
                                 start=(k == 0), stop=(k == KT - 1))
            sc_s = sp.tile([80, 1], F32, tag="sc_s")
            nc.vector.tensor_copy(sc_s[:], ps_sc[:])

            sqq = sp.tile([C_SH, 1], F32, tag="sqq")
            nc.scalar.activation(sqq[:], sc_s[64:64 + C_SH, :], AF.Sqrt)
            shat = sp.tile([C_SH, 1], F32, tag="shat")
            nc.vector.reciprocal(shat[:], sqq[:])

            ps_g1 = ps_acc.tile([C_SH, B_I], F32, tag="acc1")
            for k in range(KT):
                nc.tensor.matmul(ps_g1[:], aT2[:, k, :], bT[:, k, :],
                                 start=(k == 0), stop=False)
            nc.tensor.matmul(ps_g1[:], ones_row32[:, 0:C_SH], nb_row[:, :],
                             start=False, stop=True)
            den = sp.tile([C_SH, B_I], F32, tag="den")
            nc.scalar.activation(den[:], ps_g1[:], AF.Sqrt,
                                 bias=sc_s[32:32 + C_SH, :])
            rden = sp.tile([C_SH, B_I], F32, tag="rden")
            nc.vector.reciprocal(rden[:], den[:])

            ps_g2 = ps_acc.tile([C_SH, B_I], F32, tag="acc2")
            for k in range(KT):
                nc.tensor.matmul(ps_g2[:], csT[:, k, :], bT[:, k, :],
                                 start=(k == 0), stop=(k == KT - 1))
            num = sp.tile([C_SH, B_I], F32, tag="num")
            nc.vector.tensor_scalar(
                out=num[:], in0=ps_g2[:], scalar1=sc_s[0:C_SH, :],
                scalar2=shat[:], op0=mybir.AluOpType.add,
                op1=mybir.AluOpType.mult)
            sims_s = sp.tile([C_SH, B_I], F32, tag="sims_s")
            nc.vector.tensor_mul(sims_s[:], num[:], rden[:])
            nc.sync.dma_start(out=sims_d[:, :], in_=sims_s[:])

    nc.compile()
    return nc


def _get_nc(CT: int):
    if CT not in _CACHE:
        _CACHE[CT] = _build(CT)
    return _CACHE[CT]


def _host_prep(inputs):
    cap_embed = np.asarray(inputs["cap_embed"], dtype=np.float32)
    img_embed = np.asarray(inputs["img_embed"], dtype=np.float32)
    lens = np.asarray(inputs["lens"]).astype(np.int64)
    wvt = np.ascontiguousarray(np.asarray(inputs["Wvt"], dtype=np.float32))
    wvi = np.ascontiguousarray(np.asarray(inputs["Wvi"], dtype=np.float32))
    bvt = np.asarray(inputs["bvt"], dtype=np.float32).reshape(1, D)
    bvi = np.asarray(inputs["bvi"], dtype=np.float32).reshape(1, D)
    gamma = float(np.asarray(inputs["gamma_img"]).reshape(-1)[0])

    per_core = lens.reshape(N_CORES, C_SH)
    max_tok = int(per_core.sum(axis=1).max())
    CT = max(1, -(-max_tok // 128))
    CAP_TOK = CT * 128

    om_b = np.zeros((IMG_PAD, I_SH), np.float32)
    om_b[:IMG_TOK] = np.repeat(np.eye(I_SH, dtype=np.float32), R, axis=0) / R

    in_maps = []
    for m in range(N_CORES):
        lm = per_core[m]
        cs = slice(m * C_SH, (m + 1) * C_SH)
        im = slice(m * I_SH, (m + 1) * I_SH)
        cap_sh = cap_embed[cs]                           # [16, T, D]
        cap = np.zeros((CAP_TOK, D), np.float32)
        om_a = np.zeros((CAP_TOK, C_SH), np.float32)
        om_cm = np.zeros((CAP_TOK, C_SH), np.float32)
        pos = 0
        for c in range(C_SH):
            n = int(lm[c])
            cap[pos:pos + n] = cap_sh[c, :n]
            om_a[pos:pos + n, c] = gamma / R
            om_cm[pos:pos + n, c] = 1.0
            pos += n
        img = np.zeros((IMG_PAD, D), np.float32)
        img[:IMG_TOK] = img_embed[im].reshape(IMG_TOK, D)
        in_maps.append({
            "cap": cap,
            "img": img,
            "wvt": wvt,
            "wvi": wvi,
            "bvt_row": bvt,
            "bvi_row": bvi,
            "om_a": om_a,
            "om_cm": om_cm,
            "om_b": om_b,
        })
    return in_maps, CT


def kernel(**inputs) -> np.ndarray:
    in_maps, CT = _host_prep(inputs)
    nc = _get_nc(CT)
    res = run_bass_kernel_spmd(nc, in_maps, core_ids=list(range(N_CORES)))
    sims = np.empty((B_I, B_C), np.float32)
    for m in range(N_CORES):
        sims[:, m * C_SH:(m + 1) * C_SH] = res.results[m]["sims"].T
    return sims


def run_traced(**inputs):
    """For test.py: same as kernel() but with NTFF tracing enabled."""
    in_maps, CT = _host_prep(inputs)
    nc = _get_nc(CT)
    res = run_bass_kernel_spmd(nc, in_maps, core_ids=list(range(N_CORES)),
                               trace=True)
    sims = np.empty((B_I, B_C), np.float32)
    for m in range(N_CORES):
        sims[:, m * C_SH:(m + 1) * C_SH] = res.results[m]["sims"].T
    return sims, res
